# revision 2
# baseline (speedup 1.0000x reference)
"""Causal single-head attention (B=4, S=2048, D=1024) on 8 TRN2 NeuronCores.

Sharding: data-parallel, 2 cores per batch element; 16 query blocks split
between the pair in a triangle-balanced pattern (A: 0,3,4,7,8,11,12,15 /
B: the rest) so both cores see the same multiset of padded causal key
ranges.

Two algebraic restructurings kill most of the projection work:
  * scores = (xq wq^T)(x wk^T)^T = xq (wq^T wk) x^T. W = wq^T wk is folded
    on the host (exact, fp32), so the kernel computes U^T = W^T xq^T once
    (2.15 GF) and contracts scores directly against the x^T panels already
    resident in SBUF — the K projection (4.3 GF/batch) vanishes.
  * V is tensor-parallel within the pair on its OUTPUT columns: core h
    computes V[:, h*512:(h+1)*512] = x wv_half^T only (half the work), and
    the halves are exchanged with a single 2-core AllGather via HBM
    staging. Rank-order concat puts both halves in global e-order on both
    cores, so the per-core half selection lives entirely in the wv DATA.

Per core: 9.14 GFLOP, all matmuls bf16 with fp32 PSUM accumulation:
  V_mine = x @ wv_half^T  -> stage -> AllGather (hidden under UT + scores)
  U^T    = W^T @ xq^T
  all 8 score slots (largest first): scores(s) = U^T(s-block) . x^T, causal
    mask on the last 256 cols (additive, fed as data), exp with accumulated
    row sums (no max-subtraction: |scores| <= ~11), probs transposed via
    XBAR DMA (PE for the two smallest slots),
  then all 8 AV slots: probs^T @ V -> scale by 1/rowsum -> out.

kernel(x, wq, wk, wv) takes full inputs, returns the full [4,2048,1024] output.
"""
import ml_dtypes
import numpy as np

import concourse.bacc as bacc
import concourse.mybir as mybir
import concourse.tile as tile
from concourse.bass_utils import run_bass_kernel_spmd
from concourse.masks import make_identity

F32 = mybir.dt.float32
BF16 = mybir.dt.bfloat16
AX = mybir.AxisListType
AF = mybir.ActivationFunctionType

P = 128
B, S, D = 4, 2048, 1024
NQ = 1024            # query rows per core
EH = 512             # V e-half computed locally per core
BLOCKS_A = [0, 3, 4, 7, 8, 11, 12, 15]
BLOCKS_B = [1, 2, 5, 6, 9, 10, 13, 14]
NEG = -1e30
SCALE = 1.0 / 32.0   # 1/sqrt(D)
PAIRS = [[0, 1], [2, 3], [4, 5], [6, 7]]

_CACHE = {}


def _build():
    nc = bacc.Bacc("TRN2", target_bir_lowering=False, debug=False, num_devices=8)
    # x chunks pre-paneled host-side: [ck, p, dt, s] = x[ck*512+s, dt*128+p]
    xTP_d = nc.declare_dram_parameter("xTP", [4, P, 8, 512], BF16, isOutput=False)
    xqTP_d = nc.declare_dram_parameter("xqTP", [2, P, 8, 512], BF16, isOutput=False)
    # folded-W lhsT panels: [et, p, dt, es] = W[dt*128+p, et*128+es]
    Wp_d = nc.declare_dram_parameter("Wp", [8, P, 8, P], BF16, isOutput=False)
    # wv half rhs panels: [p, dt, e'] = wv[h*512+e', dt*128+p]
    wvh_d = nc.declare_dram_parameter("wvh", [P, 8, EH], BF16, isOutput=False)
    mb_d = nc.declare_dram_parameter("maskb", [P, 8, 256], F32, isOutput=False)
    out_d = nc.declare_dram_parameter("out", [NQ, D], F32, isOutput=True)

    # HBM staging for the pairwise V e-half exchange
    v_stage = nc.dram_tensor("v_stage", [P, 16 * EH], BF16)
    v_gath = nc.dram_tensor("v_gath", [2 * P, 16 * EH], BF16)

    with tile.TileContext(nc) as tc:
        with (
            tc.tile_pool(name="store", bufs=1) as store,
            tc.tile_pool(name="whalf", bufs=1) as whalf,
            tc.tile_pool(name="kvmine", bufs=1) as kvmine,
            tc.tile_pool(name="wqpool", bufs=3) as wqpool,
            tc.tile_pool(name="xpool", bufs=6) as xpool,
            tc.tile_pool(name="prpool", bufs=10) as prpool,
            tc.tile_pool(name="small", bufs=4) as smallp,
            tc.tile_pool(name="outp", bufs=3) as outp,
            tc.tile_pool(name="ps_a", bufs=1, space="PSUM") as psA,
            tc.tile_pool(name="ps_b", bufs=5, space="PSUM") as psB,
            tc.tile_pool(name="ps_t", bufs=2, space="PSUM") as psT,
        ):
            ident = store.tile([P, P], BF16)
            make_identity(nc, ident[:])
            V = store.tile([P, 2, 16, EH], BF16)  # [k%128, e-half, k//128, e%512]
            UT = store.tile([P, 8, NQ], BF16)   # [e%128, e//128, q]

            # ---- input DMAs in first-use order ----
            wvh = whalf.tile([P, 8, EH], BF16, tag="wv")
            for dt in range(8):
                nc.scalar.dma_start(wvh[:, dt, :], wvh_d.ap()[:, dt, :])
            xts = []
            for ck in range(4):
                xt = xpool.tile([P, 8, 512], BF16, tag="x")
                if ck == 0:
                    for dt in range(8):
                        nc.sync.dma_start(xt[:, dt, :], xTP_d.ap()[ck][:, dt, :])
                else:
                    nc.sync.dma_start(xt[:, 0:4], xTP_d.ap()[ck][:, 0:4])
                    nc.sync.dma_start(xt[:, 4:8], xTP_d.ap()[ck][:, 4:8])
                xts.append(xt)

            # ---- V_mine = x @ wv_half^T  ([P, 16, EH], e-half h) ----
            nc.enter_named_scope("p1_v", notify=False)
            Vm = kvmine.tile([P, 16, EH], BF16, tag="v")
            for ck in range(4):
                xt = xts[ck]
                for sub in range(4):
                    ps = psB.tile([P, 512], F32, tag="pb")
                    for dt in range(8):
                        nc.tensor.matmul(
                            ps[:],
                            xt[:, dt, sub * P:(sub + 1) * P],
                            wvh[:, dt, :],
                            start=(dt == 0),
                            stop=(dt == 7),
                        )
                    nc.any.tensor_copy(Vm[:, ck * 4 + sub, :], ps[:])
                nc.scalar.dma_start(
                    v_stage[:, ck * 4 * EH:(ck + 1) * 4 * EH],
                    Vm[:, ck * 4:(ck + 1) * 4, :],
                )
            nc.leave_named_scope("p1_v", 0, notify=False)
            nc.gpsimd.collective_compute(
                "AllGather",
                mybir.AluOpType.bypass,
                replica_groups=PAIRS,
                ins=[v_stage[:].opt()],
                outs=[v_gath[:].opt()],
            )
            # unpack on the gpsimd (SWDGE) ring — it has nothing else to do,
            # and with a contiguous destination the descriptor count is tiny
            nc.gpsimd.dma_start(V[:, 0], v_gath[0:P, :])
            nc.gpsimd.dma_start(V[:, 1], v_gath[P:2 * P, :])

            # ---- UT = W^T @ xq^T (et-outer, streamed W panels) ----
            nc.enter_named_scope("p1_ut", notify=False)
            xqs = []
            for cq in range(2):
                xq = xpool.tile([P, 8, 512], BF16, tag="x")
                nc.sync.dma_start(xq[:], xqTP_d.ap()[cq])
                xqs.append(xq)
            maskt = store.tile([P, 8, 256], F32)
            for et in range(8):
                wqp = wqpool.tile([P, 8, P], BF16, tag="wq")
                nc.sync.dma_start(wqp[:], Wp_d.ap()[et])
                if et == 0:
                    nc.sync.dma_start(maskt[:], mb_d.ap())
                for cq in range(2):
                    ps = psB.tile([P, 512], F32, tag="pb")
                    for dt in range(8):
                        nc.tensor.matmul(
                            ps[:],
                            wqp[:, dt, :],
                            xqs[cq][:, dt, :],
                            start=(dt == 0),
                            stop=(dt == 7),
                        )
                    nc.any.tensor_copy(UT[:, et, cq * 512:(cq + 1) * 512], ps[:])
            nc.leave_named_scope("p1_ut", 0, notify=False)

            # ---- phase 2: all score slots first (they only need x^T + UT),
            # then all AV slots — the V gather finishes under the scores. ----
            def emit_av(s, probsT, rec, ltiles):
                for ev in range(2):
                    pav = psB.tile([P, 512], F32, tag="pb")
                    for t in range(ltiles):
                        nc.tensor.matmul(
                            pav[:],
                            probsT[:, t, :],
                            V[:, ev, t, :],
                            start=(t == 0),
                            stop=(t == ltiles - 1),
                        )
                    ot = outp.tile([P, 512], F32, tag="out")
                    nc.scalar.activation(ot[:], pav[:], AF.Copy, scale=rec[:])
                    nc.sync.dma_start(
                        out_d.ap()[s * P:(s + 1) * P, ev * 512:(ev + 1) * 512],
                        ot[:],
                    )

            nc.enter_named_scope("ph2", notify=False)
            pendings = []
            for s in range(7, -1, -1):
                ltiles = 2 * (s + 1)           # 128-wide key tiles
                keys = 256 * (s + 1)
                n512 = (s + 1) // 2            # full 512-wide chunks
                rem = (s + 1) % 2              # one trailing 256-wide chunk?
                nch = n512 + rem
                probs = prpool.tile([P, S], BF16, tag="pr")
                probsT = prpool.tile([P, 16, P], BF16, tag="pr")
                sums = smallp.tile([P, 8], F32, tag="sums")
                for c in range(nch):
                    is_rem = rem and c == nch - 1
                    w = 256 if is_rem else 512
                    lo = c * 512
                    if is_rem:
                        ps = psA.tile([P, 256], F32, tag="pa")
                    else:
                        ps = psB.tile([P, 512], F32, tag="pb")
                    for dt in range(8):
                        nc.tensor.matmul(
                            ps[:],
                            UT[:, dt, s * P:(s + 1) * P],
                            xts[c][:, dt, 0:w],
                            start=(dt == 0),
                            stop=(dt == 7),
                        )
                    if lo + w == keys:   # mask the last 256 key columns
                        nc.vector.tensor_add(
                            ps[:, w - 256:w], ps[:, w - 256:w], maskt[:, s, :]
                        )
                    nc.scalar.activation(
                        probs[:, lo:lo + w],
                        ps[:],
                        AF.Exp,
                        scale=SCALE,
                        accum_out=sums[:, c:c + 1],
                    )
                    # PE transpose: the PE has slack here, and keeping the
                    # ACT ring free lets exp drain PSUM without queueing
                    # behind XBAR transposes
                    for t in range(lo // P, (lo + w) // P):
                        pt = psT.tile([P, P], BF16, tag="tr")
                        nc.tensor.transpose(
                            pt[:], probs[:, t * P:(t + 1) * P], ident[:]
                        )
                        nc.vector.tensor_copy(probsT[:, t, :], pt[:])
                den = smallp.tile([P, 1], F32, tag="den")
                nc.vector.reduce_sum(den[:], sums[:, :nch], axis=AX.X)
                rec = smallp.tile([P, 1], F32, tag="rec")
                nc.vector.reciprocal(rec[:], den[:])
                pendings.append((s, probsT, rec, ltiles))
            for p_ in pendings:
                emit_av(*p_)
            nc.leave_named_scope("ph2", 0, notify=False)
    nc.compile()
    return nc


def _make_masks():
    masks = []
    for blocks in (BLOCKS_A, BLOCKS_B):
        m = np.zeros((P, 8, 256), np.float32)
        for s, j in enumerate(blocks):
            q = j * P + np.arange(P)[:, None]
            k = 256 * s + np.arange(256)[None, :]
            m[:, s, :] = np.where(k <= q, 0.0, NEG)
        masks.append(m)
    return masks


def _bf16(a):
    return np.ascontiguousarray(a.astype(ml_dtypes.bfloat16))


def _panelize(wT):
    # wT: [D, D] = [dt*128+p, et*128+es] -> [et, p, dt, es]
    return _bf16(wT.reshape(8, P, 8, P).transpose(2, 1, 0, 3))


def _chunk_panels(rows, nck):
    # [nck*512, 1024] -> [ck, p, dt, s] with [ck,p,dt,s] = rows[ck*512+s, dt*128+p]
    return _bf16(rows.reshape(nck, 512, 8, P).transpose(0, 3, 2, 1))


LAST_RESULT = None


def kernel(x, wq, wk, wv):
    global LAST_RESULT
    x = np.ascontiguousarray(np.asarray(x, dtype=np.float32))
    wq = np.asarray(wq, dtype=np.float32)
    wk = np.asarray(wk, dtype=np.float32)
    wv = np.asarray(wv, dtype=np.float32)

    if "nc" not in _CACHE:
        _CACHE["nc"] = _build()
        _CACHE["masks"] = _make_masks()
    nc = _CACHE["nc"]
    masks = _CACHE["masks"]

    # exact host-side fold: scores = xq (wq^T wk) x^T
    W = wq.T @ wk
    Wp = _panelize(W)
    # per-half wv rhs panels: [p, dt, e'] = wv[h*512+e', dt*128+p]
    wvhs = [
        _bf16(wv[h * EH:(h + 1) * EH].T.reshape(8, P, EH).transpose(1, 0, 2))
        for h in range(2)
    ]

    in_maps = []
    for c in range(8):
        b, h = divmod(c, 2)
        blocks = BLOCKS_A if h == 0 else BLOCKS_B
        xb = x[b]
        xq = np.concatenate([xb[j * P:(j + 1) * P] for j in blocks], 0)
        in_maps.append(
            {
                "xTP": _chunk_panels(xb, 4),
                "xqTP": _chunk_panels(xq, 2),
                "Wp": Wp,
                "wvh": wvhs[h],
                "maskb": masks[h],
            }
        )

    res = run_bass_kernel_spmd(nc, in_maps, core_ids=list(range(8)))
    LAST_RESULT = res

    out = np.empty((B, S, D), np.float32)
    for c in range(8):
        b, h = divmod(c, 2)
        blocks = BLOCKS_A if h == 0 else BLOCKS_B
        oc = res.results[c]["out"]
        for si, j in enumerate(blocks):
            out[b, j * P:(j + 1) * P] = oc[si * P:(si + 1) * P]
    return out


# revision 3
# speedup vs baseline: 1.0148x; 1.0148x over previous
"""Causal single-head attention (B=4, S=2048, D=1024) on 8 TRN2 NeuronCores.

Sharding: data-parallel, 2 cores per batch element; 16 query blocks split
between the pair in a triangle-balanced pattern (A: 0,3,4,7,8,11,12,15 /
B: the rest) so both cores see the same multiset of padded causal key
ranges.

Two algebraic restructurings kill most of the projection work:
  * scores = (xq wq^T)(x wk^T)^T = xq (wq^T wk) x^T. W = wq^T wk is folded
    on the host (exact, fp32), so the kernel computes U^T = W^T xq^T once
    (2.15 GF) and contracts scores directly against the x^T panels already
    resident in SBUF — the K projection (4.3 GF/batch) vanishes.
  * V is tensor-parallel within the pair on its OUTPUT columns: core h
    computes V[:, h*512:(h+1)*512] = x wv_half^T only (half the work), and
    the halves are exchanged with a single 2-core AllGather via HBM
    staging. Rank-order concat puts both halves in global e-order on both
    cores, so the per-core half selection lives entirely in the wv DATA.

Per core: 9.14 GFLOP, all matmuls bf16 with fp32 PSUM accumulation:
  V_mine = x @ wv_half^T  -> stage -> AllGather (hidden under UT + scores)
  U^T    = W^T @ xq^T
  all 8 score slots (largest first): scores(s) = U^T(s-block) . x^T, causal
    mask on the last 256 cols (additive, fed as data), exp with accumulated
    row sums (no max-subtraction: |scores| <= ~11), probs transposed via
    XBAR DMA (PE for the two smallest slots),
  then all 8 AV slots: probs^T @ V -> scale by 1/rowsum -> out.

kernel(x, wq, wk, wv) takes full inputs, returns the full [4,2048,1024] output.
"""
import ml_dtypes
import numpy as np

import concourse.bacc as bacc
import concourse.mybir as mybir
import concourse.tile as tile
from concourse.bass_utils import run_bass_kernel_spmd
from concourse.masks import make_identity

F32 = mybir.dt.float32
FP8 = mybir.dt.float8e4
BF16 = mybir.dt.bfloat16
AX = mybir.AxisListType
AF = mybir.ActivationFunctionType

P = 128
B, S, D = 4, 2048, 1024
NQ = 1024            # query rows per core
EH = 512             # V e-half computed locally per core
BLOCKS_A = [0, 3, 4, 7, 8, 11, 12, 15]
BLOCKS_B = [1, 2, 5, 6, 9, 10, 13, 14]
NEG = -1e30
SCALE = 1.0 / 32.0   # 1/sqrt(D)
PAIRS = [[0, 1], [2, 3], [4, 5], [6, 7]]

_CACHE = {}


def _build():
    nc = bacc.Bacc("TRN2", target_bir_lowering=False, debug=False, num_devices=8)
    # x chunks pre-paneled host-side: [ck, p, dt, s] = x[ck*512+s, dt*128+p]
    xTP_d = nc.declare_dram_parameter("xTP", [4, P, 8, 512], BF16, isOutput=False)
    xT8_d = nc.declare_dram_parameter("xT8", [4, P, 4, 2, 512], FP8, isOutput=False)
    xqTP_d = nc.declare_dram_parameter("xqTP", [2, P, 8, 512], BF16, isOutput=False)
    # folded-W lhsT panels: [et, p, dt, es] = W[dt*128+p, et*128+es]
    Wp_d = nc.declare_dram_parameter("Wp", [8, P, 8, P], BF16, isOutput=False)
    # wv half rhs panels: [p, dt, e'] = wv[h*512+e', dt*128+p]
    wvh_d = nc.declare_dram_parameter("wvh", [P, 8, EH], BF16, isOutput=False)
    mb_d = nc.declare_dram_parameter("maskb", [P, 8, 256], F32, isOutput=False)
    out_d = nc.declare_dram_parameter("out", [NQ, D], F32, isOutput=True)

    # HBM staging for the pairwise V e-half exchange
    v_stage = nc.dram_tensor("v_stage", [P, 16 * EH], BF16)
    v_gath = nc.dram_tensor("v_gath", [2 * P, 16 * EH], BF16)

    with tile.TileContext(nc) as tc:
        with (
            tc.tile_pool(name="store", bufs=1) as store,
            tc.tile_pool(name="whalf", bufs=1) as whalf,
            tc.tile_pool(name="kvmine", bufs=1) as kvmine,
            tc.tile_pool(name="wqpool", bufs=3) as wqpool,
            tc.tile_pool(name="xpool", bufs=6) as xpool,
            tc.tile_pool(name="prpool", bufs=10) as prpool,
            tc.tile_pool(name="small", bufs=4) as smallp,
            tc.tile_pool(name="outp", bufs=3) as outp,
            tc.tile_pool(name="ps_a", bufs=1, space="PSUM") as psA,
            tc.tile_pool(name="ps_b", bufs=5, space="PSUM") as psB,
            tc.tile_pool(name="ps_t", bufs=2, space="PSUM") as psT,
        ):
            ident = store.tile([P, P], BF16)
            make_identity(nc, ident[:])
            V = store.tile([P, 2, 16, EH], BF16)  # [k%128, e-half, k//128, e%512]
            UT8 = store.tile([P, 4, 2, NQ], FP8)  # paired d' slabs for DoubleRow

            # ---- input DMAs in first-use order ----
            wvh = whalf.tile([P, 8, EH], BF16, tag="wv")
            for dt in range(8):
                nc.scalar.dma_start(wvh[:, dt, :], wvh_d.ap()[:, dt, :])
            xts = []
            for ck in range(4):
                xt = xpool.tile([P, 8, 512], BF16, tag="x")
                if ck == 0:
                    for dt in range(8):
                        nc.sync.dma_start(xt[:, dt, :], xTP_d.ap()[ck][:, dt, :])
                else:
                    nc.sync.dma_start(xt[:, 0:4], xTP_d.ap()[ck][:, 0:4])
                    nc.sync.dma_start(xt[:, 4:8], xTP_d.ap()[ck][:, 4:8])
                xts.append(xt)

            # ---- V_mine = x @ wv_half^T  ([P, 16, EH], e-half h) ----
            nc.enter_named_scope("p1_v", notify=False)
            Vm = kvmine.tile([P, 16, EH], BF16, tag="v")
            for ck in range(4):
                xt = xts[ck]
                for sub in range(4):
                    ps = psB.tile([P, 512], F32, tag="pb")
                    for dt in range(8):
                        nc.tensor.matmul(
                            ps[:],
                            xt[:, dt, sub * P:(sub + 1) * P],
                            wvh[:, dt, :],
                            start=(dt == 0),
                            stop=(dt == 7),
                        )
                    nc.any.tensor_copy(Vm[:, ck * 4 + sub, :], ps[:])
                nc.scalar.dma_start(
                    v_stage[:, ck * 4 * EH:(ck + 1) * 4 * EH],
                    Vm[:, ck * 4:(ck + 1) * 4, :],
                )
            nc.leave_named_scope("p1_v", 0, notify=False)
            nc.gpsimd.collective_compute(
                "AllGather",
                mybir.AluOpType.bypass,
                replica_groups=PAIRS,
                ins=[v_stage[:].opt()],
                outs=[v_gath[:].opt()],
            )
            # unpack on the gpsimd (SWDGE) ring — it has nothing else to do,
            # and with a contiguous destination the descriptor count is tiny
            nc.gpsimd.dma_start(V[:, 0], v_gath[0:P, :])
            nc.gpsimd.dma_start(V[:, 1], v_gath[P:2 * P, :])

            # ---- UT = W^T @ xq^T (et-outer, streamed W panels) ----
            nc.enter_named_scope("p1_ut", notify=False)
            xt8s = []
            for ck in range(4):
                x8 = xpool.tile([P, 4, 2, 512], FP8, tag="x8")
                nc.sync.dma_start(x8[:], xT8_d.ap()[ck])
                xt8s.append(x8)
            xqs = []
            for cq in range(2):
                xq = xpool.tile([P, 8, 512], BF16, tag="x")
                nc.sync.dma_start(xq[:], xqTP_d.ap()[cq])
                xqs.append(xq)
            maskt = store.tile([P, 8, 256], F32)
            for et in range(8):
                wqp = wqpool.tile([P, 8, P], BF16, tag="wq")
                nc.sync.dma_start(wqp[:], Wp_d.ap()[et])
                if et == 0:
                    nc.sync.dma_start(maskt[:], mb_d.ap())
                for cq in range(2):
                    ps = psB.tile([P, 512], F32, tag="pb")
                    for dt in range(8):
                        nc.tensor.matmul(
                            ps[:],
                            wqp[:, dt, :],
                            xqs[cq][:, dt, :],
                            start=(dt == 0),
                            stop=(dt == 7),
                        )
                    nc.any.tensor_copy(
                        UT8[:, et // 2, et % 2, cq * 512:(cq + 1) * 512], ps[:]
                    )
            nc.leave_named_scope("p1_ut", 0, notify=False)

            # ---- phase 2: all score slots first (they only need x^T + UT),
            # then all AV slots — the V gather finishes under the scores. ----
            def emit_av(s, probsT, rec, ltiles):
                for ev in range(2):
                    pav = psB.tile([P, 512], F32, tag="pb")
                    for t in range(ltiles):
                        nc.tensor.matmul(
                            pav[:],
                            probsT[:, t, :],
                            V[:, ev, t, :],
                            start=(t == 0),
                            stop=(t == ltiles - 1),
                        )
                    ot = outp.tile([P, 512], F32, tag="out")
                    nc.scalar.activation(ot[:], pav[:], AF.Copy, scale=rec[:])
                    nc.sync.dma_start(
                        out_d.ap()[s * P:(s + 1) * P, ev * 512:(ev + 1) * 512],
                        ot[:],
                    )

            nc.enter_named_scope("ph2", notify=False)
            pendings = []
            for s in range(7, -1, -1):
                ltiles = 2 * (s + 1)           # 128-wide key tiles
                keys = 256 * (s + 1)
                n512 = (s + 1) // 2            # full 512-wide chunks
                rem = (s + 1) % 2              # one trailing 256-wide chunk?
                nch = n512 + rem
                probs = prpool.tile([P, S], BF16, tag="pr")
                probsT = prpool.tile([P, 16, P], BF16, tag="pr")
                sums = smallp.tile([P, 8], F32, tag="sums")
                for c in range(nch):
                    is_rem = rem and c == nch - 1
                    w = 256 if is_rem else 512
                    lo = c * 512
                    if is_rem:
                        ps = psA.tile([P, 256], F32, tag="pa")
                    else:
                        ps = psB.tile([P, 512], F32, tag="pb")
                    for pt in range(4):
                        nc.tensor.matmul(
                            ps[:],
                            UT8[:, pt, :, s * P:(s + 1) * P],
                            xt8s[c][:, pt, :, 0:w],
                            start=(pt == 0),
                            stop=(pt == 3),
                            perf_mode=mybir.MatmulPerfMode.DoubleRow,
                        )
                    if lo + w == keys:   # mask the last 256 key columns
                        nc.vector.tensor_add(
                            ps[:, w - 256:w], ps[:, w - 256:w], maskt[:, s, :]
                        )
                    nc.scalar.activation(
                        probs[:, lo:lo + w],
                        ps[:],
                        AF.Exp,
                        scale=SCALE,
                        accum_out=sums[:, c:c + 1],
                    )
                    # PE transpose: the PE has slack here, and keeping the
                    # ACT ring free lets exp drain PSUM without queueing
                    # behind XBAR transposes
                    for t in range(lo // P, (lo + w) // P):
                        pt = psT.tile([P, P], BF16, tag="tr")
                        nc.tensor.transpose(
                            pt[:], probs[:, t * P:(t + 1) * P], ident[:]
                        )
                        nc.vector.tensor_copy(probsT[:, t, :], pt[:])
                den = smallp.tile([P, 1], F32, tag="den")
                nc.vector.reduce_sum(den[:], sums[:, :nch], axis=AX.X)
                rec = smallp.tile([P, 1], F32, tag="rec")
                nc.vector.reciprocal(rec[:], den[:])
                pendings.append((s, probsT, rec, ltiles))
            for p_ in pendings:
                emit_av(*p_)
            nc.leave_named_scope("ph2", 0, notify=False)
    nc.compile()
    return nc


def _make_masks():
    masks = []
    for blocks in (BLOCKS_A, BLOCKS_B):
        m = np.zeros((P, 8, 256), np.float32)
        for s, j in enumerate(blocks):
            q = j * P + np.arange(P)[:, None]
            k = 256 * s + np.arange(256)[None, :]
            m[:, s, :] = np.where(k <= q, 0.0, NEG)
        masks.append(m)
    return masks


def _bf16(a):
    return np.ascontiguousarray(a.astype(ml_dtypes.bfloat16))


def _panelize(wT):
    # wT: [D, D] = [dt*128+p, et*128+es] -> [et, p, dt, es]
    return _bf16(wT.reshape(8, P, 8, P).transpose(2, 1, 0, 3))


def _chunk_panels(rows, nck):
    # [nck*512, 1024] -> [ck, p, dt, s] with [ck,p,dt,s] = rows[ck*512+s, dt*128+p]
    return _bf16(rows.reshape(nck, 512, 8, P).transpose(0, 3, 2, 1))


def _chunk_panels8(rows):
    # [2048, 1024] -> [ck, p, pt, i, s] = fp8(rows[ck*512+s, (2*pt+i)*128+p])
    a = rows.reshape(4, 512, 4, 2, P).transpose(0, 4, 2, 3, 1)
    return np.ascontiguousarray(a.astype(ml_dtypes.float8_e4m3fn))


LAST_RESULT = None


def kernel(x, wq, wk, wv):
    global LAST_RESULT
    x = np.ascontiguousarray(np.asarray(x, dtype=np.float32))
    wq = np.asarray(wq, dtype=np.float32)
    wk = np.asarray(wk, dtype=np.float32)
    wv = np.asarray(wv, dtype=np.float32)

    if "nc" not in _CACHE:
        _CACHE["nc"] = _build()
        _CACHE["masks"] = _make_masks()
    nc = _CACHE["nc"]
    masks = _CACHE["masks"]

    # exact host-side fold: scores = xq (wq^T wk) x^T
    W = wq.T @ wk
    Wp = _panelize(W)
    # per-half wv rhs panels: [p, dt, e'] = wv[h*512+e', dt*128+p]
    wvhs = [
        _bf16(wv[h * EH:(h + 1) * EH].T.reshape(8, P, EH).transpose(1, 0, 2))
        for h in range(2)
    ]

    in_maps = []
    for c in range(8):
        b, h = divmod(c, 2)
        blocks = BLOCKS_A if h == 0 else BLOCKS_B
        xb = x[b]
        xq = np.concatenate([xb[j * P:(j + 1) * P] for j in blocks], 0)
        in_maps.append(
            {
                "xTP": _chunk_panels(xb, 4),
                "xT8": _chunk_panels8(xb),
                "xqTP": _chunk_panels(xq, 2),
                "Wp": Wp,
                "wvh": wvhs[h],
                "maskb": masks[h],
            }
        )

    res = run_bass_kernel_spmd(nc, in_maps, core_ids=list(range(8)))
    LAST_RESULT = res

    out = np.empty((B, S, D), np.float32)
    for c in range(8):
        b, h = divmod(c, 2)
        blocks = BLOCKS_A if h == 0 else BLOCKS_B
        oc = res.results[c]["out"]
        for si, j in enumerate(blocks):
            out[b, j * P:(j + 1) * P] = oc[si * P:(si + 1) * P]
    return out


# revision 4
# speedup vs baseline: 1.0182x; 1.0034x over previous
"""Causal single-head attention (B=4, S=2048, D=1024) on 8 TRN2 NeuronCores.

Sharding: data-parallel, 2 cores per batch element; 16 query blocks split
between the pair in a triangle-balanced pattern (A: 0,3,4,7,8,11,12,15 /
B: the rest) so both cores see the same multiset of padded causal key
ranges.

Two algebraic restructurings kill most of the projection work:
  * scores = (xq wq^T)(x wk^T)^T = xq (wq^T wk) x^T. W = wq^T wk is folded
    on the host (exact, fp32), so the kernel computes U^T = W^T xq^T once
    (2.15 GF) and contracts scores directly against the x^T panels already
    resident in SBUF — the K projection (4.3 GF/batch) vanishes.
  * V is tensor-parallel within the pair on its OUTPUT columns: core h
    computes V[:, h*512:(h+1)*512] = x wv_half^T only (half the work), and
    the halves are exchanged with a single 2-core AllGather via HBM
    staging. Rank-order concat puts both halves in global e-order on both
    cores, so the per-core half selection lives entirely in the wv DATA.

Per core: 9.14 GFLOP, all matmuls bf16 with fp32 PSUM accumulation:
  V_mine = x @ wv_half^T  -> stage -> AllGather (hidden under UT + scores)
  U^T    = W^T @ xq^T
  all 8 score slots (largest first): scores(s) = U^T(s-block) . x^T, causal
    mask on the last 256 cols (additive, fed as data), exp with accumulated
    row sums (no max-subtraction: |scores| <= ~11), probs transposed via
    XBAR DMA (PE for the two smallest slots),
  then all 8 AV slots: probs^T @ V -> scale by 1/rowsum -> out.

kernel(x, wq, wk, wv) takes full inputs, returns the full [4,2048,1024] output.
"""
import ml_dtypes
import numpy as np

import concourse.bacc as bacc
import concourse.mybir as mybir
import concourse.tile as tile
from concourse.bass_utils import run_bass_kernel_spmd
from concourse.masks import make_identity

F32 = mybir.dt.float32
BF16 = mybir.dt.bfloat16
AX = mybir.AxisListType
AF = mybir.ActivationFunctionType

P = 128
B, S, D = 4, 2048, 1024
NQ = 1024            # query rows per core
EH = 512             # V e-half computed locally per core
BLOCKS_A = [0, 3, 4, 7, 8, 11, 12, 15]
BLOCKS_B = [1, 2, 5, 6, 9, 10, 13, 14]
NEG = -1e30
SCALE = 1.0 / 32.0   # 1/sqrt(D)
PAIRS = [[0, 1], [2, 3], [4, 5], [6, 7]]

_CACHE = {}


def _build():
    nc = bacc.Bacc("TRN2", target_bir_lowering=False, debug=False, num_devices=8)
    # x chunks pre-paneled host-side: [ck, p, dt, s] = x[ck*512+s, dt*128+p]
    xTP_d = nc.declare_dram_parameter("xTP", [4, P, 8, 512], BF16, isOutput=False)
    xqTP_d = nc.declare_dram_parameter("xqTP", [2, P, 8, 512], BF16, isOutput=False)
    # folded-W lhsT panels: [et, p, dt, es] = W[dt*128+p, et*128+es]
    Wp_d = nc.declare_dram_parameter("Wp", [8, P, 8, P], BF16, isOutput=False)
    # wv half rhs panels: [p, dt, e'] = wv[h*512+e', dt*128+p]
    wvh_d = nc.declare_dram_parameter("wvh", [P, 8, EH], BF16, isOutput=False)
    mb_d = nc.declare_dram_parameter("maskb", [P, 8, 256], F32, isOutput=False)
    out_d = nc.declare_dram_parameter("out", [NQ, D], F32, isOutput=True)

    # HBM staging for the pairwise V e-half exchange
    v_stage = nc.dram_tensor("v_stage", [P, 16 * EH], BF16)
    v_gath = nc.dram_tensor("v_gath", [2 * P, 16 * EH], BF16)

    with tile.TileContext(nc) as tc:
        with (
            tc.tile_pool(name="store", bufs=1) as store,
            tc.tile_pool(name="whalf", bufs=1) as whalf,
            tc.tile_pool(name="kvmine", bufs=1) as kvmine,
            tc.tile_pool(name="wqpool", bufs=3) as wqpool,
            tc.tile_pool(name="xpool", bufs=6) as xpool,
            tc.tile_pool(name="prpool", bufs=10) as prpool,
            tc.tile_pool(name="small", bufs=4) as smallp,
            tc.tile_pool(name="outp", bufs=3) as outp,
            tc.tile_pool(name="ps_a", bufs=1, space="PSUM") as psA,
            tc.tile_pool(name="ps_b", bufs=5, space="PSUM") as psB,
            tc.tile_pool(name="ps_t", bufs=2, space="PSUM") as psT,
        ):
            ident = store.tile([P, P], BF16)
            make_identity(nc, ident[:])
            V = store.tile([P, 2, 16, EH], BF16)  # [k%128, e-half, k//128, e%512]
            UT = store.tile([P, 8, NQ], BF16)   # [e%128, e//128, q]

            # ---- input DMAs in first-use order ----
            wvh = whalf.tile([P, 8, EH], BF16, tag="wv")
            for dt in range(8):
                nc.scalar.dma_start(wvh[:, dt, :], wvh_d.ap()[:, dt, :])
            xts = []
            for ck in range(4):
                xt = xpool.tile([P, 8, 512], BF16, tag="x")
                if ck == 0:
                    for dt in range(8):
                        nc.sync.dma_start(xt[:, dt, :], xTP_d.ap()[ck][:, dt, :])
                else:
                    nc.sync.dma_start(xt[:, 0:4], xTP_d.ap()[ck][:, 0:4])
                    nc.sync.dma_start(xt[:, 4:8], xTP_d.ap()[ck][:, 4:8])
                xts.append(xt)

            # ---- V_mine = x @ wv_half^T  ([P, 16, EH], e-half h) ----
            nc.enter_named_scope("p1_v", notify=False)
            Vm = kvmine.tile([P, 16, EH], BF16, tag="v")
            for ck in range(4):
                xt = xts[ck]
                for sub in range(4):
                    ps = psB.tile([P, 512], F32, tag="pb")
                    for dt in range(8):
                        nc.tensor.matmul(
                            ps[:],
                            xt[:, dt, sub * P:(sub + 1) * P],
                            wvh[:, dt, :],
                            start=(dt == 0),
                            stop=(dt == 7),
                        )
                    nc.any.tensor_copy(Vm[:, ck * 4 + sub, :], ps[:])
                nc.scalar.dma_start(
                    v_stage[:, ck * 4 * EH:(ck + 1) * 4 * EH],
                    Vm[:, ck * 4:(ck + 1) * 4, :],
                )
            nc.leave_named_scope("p1_v", 0, notify=False)
            nc.gpsimd.collective_compute(
                "AllGather",
                mybir.AluOpType.bypass,
                replica_groups=PAIRS,
                ins=[v_stage[:].opt()],
                outs=[v_gath[:].opt()],
            )
            # unpack on the gpsimd (SWDGE) ring — it has nothing else to do,
            # and with a contiguous destination the descriptor count is tiny
            nc.gpsimd.dma_start(V[:, 0], v_gath[0:P, :])
            nc.gpsimd.dma_start(V[:, 1], v_gath[P:2 * P, :])

            # ---- UT = W^T @ xq^T (et-outer, streamed W panels) ----
            nc.enter_named_scope("p1_ut", notify=False)
            xqs = []
            for cq in range(2):
                xq = xpool.tile([P, 8, 512], BF16, tag="x")
                nc.sync.dma_start(xq[:], xqTP_d.ap()[cq])
                xqs.append(xq)
            maskt = store.tile([P, 8, 256], F32)
            for et in range(8):
                wqp = wqpool.tile([P, 8, P], BF16, tag="wq")
                nc.sync.dma_start(wqp[:], Wp_d.ap()[et])
                if et == 0:
                    nc.sync.dma_start(maskt[:], mb_d.ap())
                for cq in range(2):
                    ps = psB.tile([P, 512], F32, tag="pb")
                    for dt in range(8):
                        nc.tensor.matmul(
                            ps[:],
                            wqp[:, dt, :],
                            xqs[cq][:, dt, :],
                            start=(dt == 0),
                            stop=(dt == 7),
                        )
                    nc.any.tensor_copy(UT[:, et, cq * 512:(cq + 1) * 512], ps[:])
            nc.leave_named_scope("p1_ut", 0, notify=False)

            # ---- phase 2: all score slots first (they only need x^T + UT),
            # then all AV slots — the V gather finishes under the scores. ----
            def emit_av(s, probsT, rec, ltiles):
                for ev in range(2):
                    pav = psB.tile([P, 512], F32, tag="pb")
                    for t in range(ltiles):
                        nc.tensor.matmul(
                            pav[:],
                            probsT[:, t, :],
                            V[:, ev, t, :],
                            start=(t == 0),
                            stop=(t == ltiles - 1),
                        )
                    ot = outp.tile([P, 512], F32, tag="out")
                    nc.scalar.activation(ot[:], pav[:], AF.Copy, scale=rec[:])
                    nc.sync.dma_start(
                        out_d.ap()[s * P:(s + 1) * P, ev * 512:(ev + 1) * 512],
                        ot[:],
                    )

            nc.enter_named_scope("ph2", notify=False)
            pendings = []
            for s in range(7, -1, -1):
                ltiles = 2 * (s + 1)           # 128-wide key tiles
                keys = 256 * (s + 1)
                n512 = (s + 1) // 2            # full 512-wide chunks
                rem = (s + 1) % 2              # one trailing 256-wide chunk?
                nch = n512 + rem
                probs = prpool.tile([P, S], BF16, tag="pr")
                probsT = prpool.tile([P, 16, P], BF16, tag="pr")
                sums = smallp.tile([P, 8], F32, tag="sums")
                for c in range(nch):
                    is_rem = rem and c == nch - 1
                    w = 256 if is_rem else 512
                    lo = c * 512
                    if is_rem:
                        ps = psA.tile([P, 256], F32, tag="pa")
                    else:
                        ps = psB.tile([P, 512], F32, tag="pb")
                    for dt in range(8):
                        nc.tensor.matmul(
                            ps[:],
                            UT[:, dt, s * P:(s + 1) * P],
                            xts[c][:, dt, 0:w],
                            start=(dt == 0),
                            stop=(dt == 7),
                        )
                    if lo + w == keys:   # mask the last 256 key columns
                        nc.vector.tensor_add(
                            ps[:, w - 256:w], ps[:, w - 256:w], maskt[:, s, :]
                        )
                    nc.scalar.activation(
                        probs[:, lo:lo + w],
                        ps[:],
                        AF.Exp,
                        scale=SCALE,
                        accum_out=sums[:, c:c + 1],
                    )
                    # PE transpose: the PE has slack here, and keeping the
                    # ACT ring free lets exp drain PSUM without queueing
                    # behind XBAR transposes
                    for t in range(lo // P, (lo + w) // P):
                        pt = psT.tile([P, P], BF16, tag="tr")
                        nc.tensor.transpose(
                            pt[:], probs[:, t * P:(t + 1) * P], ident[:]
                        )
                        nc.vector.tensor_copy(probsT[:, t, :], pt[:])
                den = smallp.tile([P, 1], F32, tag="den")
                nc.vector.reduce_sum(den[:], sums[:, :nch], axis=AX.X)
                rec = smallp.tile([P, 1], F32, tag="rec")
                nc.vector.reciprocal(rec[:], den[:])
                pendings.append((s, probsT, rec, ltiles))
            for p_ in pendings:
                emit_av(*p_)
            nc.leave_named_scope("ph2", 0, notify=False)
    nc.compile()
    return nc


def _make_masks():
    masks = []
    for blocks in (BLOCKS_A, BLOCKS_B):
        m = np.zeros((P, 8, 256), np.float32)
        for s, j in enumerate(blocks):
            q = j * P + np.arange(P)[:, None]
            k = 256 * s + np.arange(256)[None, :]
            m[:, s, :] = np.where(k <= q, 0.0, NEG)
        masks.append(m)
    return masks


def _bf16(a):
    return np.ascontiguousarray(a.astype(ml_dtypes.bfloat16))


def _panelize(wT):
    # wT: [D, D] = [dt*128+p, et*128+es] -> [et, p, dt, es]
    return _bf16(wT.reshape(8, P, 8, P).transpose(2, 1, 0, 3))


def _chunk_panels(rows, nck):
    # [nck*512, 1024] -> [ck, p, dt, s] with [ck,p,dt,s] = rows[ck*512+s, dt*128+p]
    return _bf16(rows.reshape(nck, 512, 8, P).transpose(0, 3, 2, 1))


LAST_RESULT = None


def kernel(x, wq, wk, wv):
    global LAST_RESULT
    x = np.ascontiguousarray(np.asarray(x, dtype=np.float32))
    wq = np.asarray(wq, dtype=np.float32)
    wk = np.asarray(wk, dtype=np.float32)
    wv = np.asarray(wv, dtype=np.float32)

    if "nc" not in _CACHE:
        _CACHE["nc"] = _build()
        _CACHE["masks"] = _make_masks()
    nc = _CACHE["nc"]
    masks = _CACHE["masks"]

    # exact host-side fold: scores = xq (wq^T wk) x^T
    W = wq.T @ wk
    Wp = _panelize(W)
    # per-half wv rhs panels: [p, dt, e'] = wv[h*512+e', dt*128+p]
    wvhs = [
        _bf16(wv[h * EH:(h + 1) * EH].T.reshape(8, P, EH).transpose(1, 0, 2))
        for h in range(2)
    ]

    in_maps = []
    for c in range(8):
        b, h = divmod(c, 2)
        blocks = BLOCKS_A if h == 0 else BLOCKS_B
        xb = x[b]
        xq = np.concatenate([xb[j * P:(j + 1) * P] for j in blocks], 0)
        in_maps.append(
            {
                "xTP": _chunk_panels(xb, 4),
                "xqTP": _chunk_panels(xq, 2),
                "Wp": Wp,
                "wvh": wvhs[h],
                "maskb": masks[h],
            }
        )

    res = run_bass_kernel_spmd(nc, in_maps, core_ids=list(range(8)))
    LAST_RESULT = res

    out = np.empty((B, S, D), np.float32)
    for c in range(8):
        b, h = divmod(c, 2)
        blocks = BLOCKS_A if h == 0 else BLOCKS_B
        oc = res.results[c]["out"]
        for si, j in enumerate(blocks):
            out[b, j * P:(j + 1) * P] = oc[si * P:(si + 1) * P]
    return out


# revision 5
# speedup vs baseline: 1.0515x; 1.0327x over previous
"""Causal single-head attention (B=4, S=2048, D=1024) on 8 TRN2 NeuronCores.

Sharding: data-parallel, 2 cores per batch element; 16 query blocks split
between the pair in a triangle-balanced pattern (A: 0,3,4,7,8,11,12,15 /
B: the rest) so both cores see the same multiset of padded causal key
ranges.

Two algebraic restructurings kill most of the projection work:
  * scores = (xq wq^T)(x wk^T)^T = xq (wq^T wk) x^T. W = wq^T wk is folded
    on the host (exact, fp32), so the kernel computes U^T = W^T xq^T once
    (2.15 GF) and contracts scores directly against the x^T panels already
    resident in SBUF — the K projection (4.3 GF/batch) vanishes.
  * V is tensor-parallel within the pair on its OUTPUT columns: core h
    computes V[:, h*512:(h+1)*512] = x wv_half^T only (half the work), and
    the halves are exchanged with a single 2-core AllGather via HBM
    staging. Rank-order concat puts both halves in global e-order on both
    cores, so the per-core half selection lives entirely in the wv DATA.

Per core: 9.14 GFLOP, all matmuls bf16 with fp32 PSUM accumulation:
  V_mine = x @ wv_half^T  -> stage -> AllGather (hidden under UT + scores)
  U^T    = W^T @ xq^T
  all 8 score slots (largest first): scores(s) = U^T(s-block) . x^T, causal
    mask on the last 256 cols (additive, fed as data), exp with accumulated
    row sums (no max-subtraction: |scores| <= ~11), probs transposed on the
    PE (the ACT engine is saturated by exp; the PE has slack here),
  then all 8 AV slots: probs^T @ V -> scale by 1/rowsum -> out.

kernel(x, wq, wk, wv) takes full inputs, returns the full [4,2048,1024] output.
"""
import ml_dtypes
import numpy as np

import concourse.bacc as bacc
import concourse.mybir as mybir
import concourse.tile as tile
from concourse.bass_utils import run_bass_kernel_spmd
from concourse.masks import make_identity

F32 = mybir.dt.float32
BF16 = mybir.dt.bfloat16
AX = mybir.AxisListType
AF = mybir.ActivationFunctionType

P = 128
B, S, D = 4, 2048, 1024
NQ = 1024            # query rows per core
EH = 512             # V e-half computed locally per core
BLOCKS_A = [0, 3, 4, 7, 8, 11, 12, 15]
BLOCKS_B = [1, 2, 5, 6, 9, 10, 13, 14]
NEG = -1e30
SCALE = 1.0 / 32.0   # 1/sqrt(D)
PAIRS = [[0, 1], [2, 3], [4, 5], [6, 7]]

_CACHE = {}


def _build():
    nc = bacc.Bacc("TRN2", target_bir_lowering=False, debug=False, num_devices=8)
    # x chunks pre-paneled host-side: [ck, p, dt, s] = x[ck*512+s, dt*128+p]
    xTP_d = nc.declare_dram_parameter("xTP", [4, P, 8, 512], BF16, isOutput=False)
    xqTP_d = nc.declare_dram_parameter("xqTP", [2, P, 8, 512], BF16, isOutput=False)
    # folded-W lhsT panels: [et, p, dt, es] = W[dt*128+p, et*128+es]
    Wp_d = nc.declare_dram_parameter("Wp", [8, P, 8, P], BF16, isOutput=False)
    # wv half rhs panels: [p, dt, e'] = wv[h*512+e', dt*128+p]
    wvh_d = nc.declare_dram_parameter("wvh", [P, 8, EH], BF16, isOutput=False)
    mb_d = nc.declare_dram_parameter("maskb", [P, 8, 256], F32, isOutput=False)
    out_d = nc.declare_dram_parameter("out", [NQ, D], F32, isOutput=True)

    # HBM staging for the pairwise V e-half exchange
    v_stage = nc.dram_tensor("v_stage", [P, 16 * EH], BF16)
    v_gath = nc.dram_tensor("v_gath", [2 * P, 16 * EH], BF16)

    with tile.TileContext(nc) as tc:
        with (
            tc.tile_pool(name="store", bufs=1) as store,
            tc.tile_pool(name="whalf", bufs=1) as whalf,
            tc.tile_pool(name="kvmine", bufs=1) as kvmine,
            tc.tile_pool(name="wqpool", bufs=3) as wqpool,
            tc.tile_pool(name="xpool", bufs=6) as xpool,
            tc.tile_pool(name="prpool", bufs=10) as prpool,
            tc.tile_pool(name="small", bufs=4) as smallp,
            tc.tile_pool(name="outp", bufs=3) as outp,
            tc.tile_pool(name="ps_a", bufs=1, space="PSUM") as psA,
            tc.tile_pool(name="ps_b", bufs=5, space="PSUM") as psB,
            tc.tile_pool(name="ps_t", bufs=2, space="PSUM") as psT,
        ):
            ident = store.tile([P, P], BF16)
            make_identity(nc, ident[:])
            V = store.tile([P, 2, 16, EH], BF16)  # [k%128, e-half, k//128, e%512]
            UT = store.tile([P, 8, NQ], BF16)   # [e%128, e//128, q]

            # ---- input DMAs in first-use order ----
            wvh = whalf.tile([P, 8, EH], BF16, tag="wv")
            for dt in range(8):
                nc.scalar.dma_start(wvh[:, dt, :], wvh_d.ap()[:, dt, :])
            xts = []
            for ck in range(4):
                xt = xpool.tile([P, 8, 512], BF16, tag="x")
                if ck == 0:
                    for dt in range(8):
                        nc.sync.dma_start(xt[:, dt, :], xTP_d.ap()[ck][:, dt, :])
                else:
                    nc.sync.dma_start(xt[:, 0:4], xTP_d.ap()[ck][:, 0:4])
                    nc.sync.dma_start(xt[:, 4:8], xTP_d.ap()[ck][:, 4:8])
                xts.append(xt)

            # ---- V_mine = x @ wv_half^T  ([P, 16, EH], e-half h) ----
            nc.enter_named_scope("p1_v", notify=False)
            Vm = kvmine.tile([P, 16, EH], BF16, tag="v")
            for ck in range(4):
                xt = xts[ck]
                for sub in range(4):
                    ps = psB.tile([P, 512], F32, tag="pb")
                    for dt in range(8):
                        nc.tensor.matmul(
                            ps[:],
                            xt[:, dt, sub * P:(sub + 1) * P],
                            wvh[:, dt, :],
                            start=(dt == 0),
                            stop=(dt == 7),
                        )
                    nc.any.tensor_copy(Vm[:, ck * 4 + sub, :], ps[:])
                nc.scalar.dma_start(
                    v_stage[:, ck * 4 * EH:(ck + 1) * 4 * EH],
                    Vm[:, ck * 4:(ck + 1) * 4, :],
                )
            nc.leave_named_scope("p1_v", 0, notify=False)
            nc.gpsimd.collective_compute(
                "AllGather",
                mybir.AluOpType.bypass,
                replica_groups=PAIRS,
                ins=[v_stage[:].opt()],
                outs=[v_gath[:].opt()],
            )
            # unpack on the gpsimd (SWDGE) ring — it has nothing else to do,
            # and with a contiguous destination the descriptor count is tiny
            nc.gpsimd.dma_start(V[:, 0], v_gath[0:P, :])
            nc.gpsimd.dma_start(V[:, 1], v_gath[P:2 * P, :])

            # ---- UT = W^T @ xq^T (et-outer, streamed W panels) ----
            nc.enter_named_scope("p1_ut", notify=False)
            xqs = []
            for cq in range(2):
                xq = xpool.tile([P, 8, 512], BF16, tag="x")
                nc.sync.dma_start(xq[:], xqTP_d.ap()[cq])
                xqs.append(xq)
            maskt = store.tile([P, 8, 256], F32)
            for et in range(8):
                wqp = wqpool.tile([P, 8, P], BF16, tag="wq")
                nc.sync.dma_start(wqp[:], Wp_d.ap()[et])
                if et == 0:
                    nc.sync.dma_start(maskt[:], mb_d.ap())
                for cq in range(2):
                    ps = psB.tile([P, 512], F32, tag="pb")
                    for dt in range(8):
                        nc.tensor.matmul(
                            ps[:],
                            wqp[:, dt, :],
                            xqs[cq][:, dt, :],
                            start=(dt == 0),
                            stop=(dt == 7),
                        )
                    nc.any.tensor_copy(UT[:, et, cq * 512:(cq + 1) * 512], ps[:])
            nc.leave_named_scope("p1_ut", 0, notify=False)

            # ---- phase 2: all score slots first (they only need x^T + UT),
            # then all AV slots — the V gather finishes under the scores. ----
            def emit_av(s, probsT, rec, ltiles):
                for ev in range(2):
                    pav = psB.tile([P, 512], F32, tag="pb")
                    for t in range(ltiles):
                        nc.tensor.matmul(
                            pav[:],
                            probsT[:, t, :],
                            V[:, ev, t, :],
                            start=(t == 0),
                            stop=(t == ltiles - 1),
                        )
                    ot = outp.tile([P, 512], F32, tag="out")
                    nc.scalar.activation(ot[:], pav[:], AF.Copy, scale=rec[:])
                    nc.sync.dma_start(
                        out_d.ap()[s * P:(s + 1) * P, ev * 512:(ev + 1) * 512],
                        ot[:],
                    )

            nc.enter_named_scope("ph2", notify=False)
            pendings = []
            for s in range(7, -1, -1):
                ltiles = 2 * (s + 1)           # 128-wide key tiles
                keys = 256 * (s + 1)
                n512 = (s + 1) // 2            # full 512-wide chunks
                rem = (s + 1) % 2              # one trailing 256-wide chunk?
                nch = n512 + rem
                probs = prpool.tile([P, S], BF16, tag="pr")
                probsT = prpool.tile([P, 16, P], BF16, tag="pr")
                sums = smallp.tile([P, 8], F32, tag="sums")
                for c in range(nch):
                    is_rem = rem and c == nch - 1
                    w = 256 if is_rem else 512
                    lo = c * 512
                    if is_rem:
                        ps = psA.tile([P, 256], F32, tag="pa")
                    else:
                        ps = psB.tile([P, 512], F32, tag="pb")
                    for dt in range(8):
                        nc.tensor.matmul(
                            ps[:],
                            UT[:, dt, s * P:(s + 1) * P],
                            xts[c][:, dt, 0:w],
                            start=(dt == 0),
                            stop=(dt == 7),
                        )
                    if lo + w == keys:   # mask the last 256 key columns
                        nc.vector.tensor_add(
                            ps[:, w - 256:w], ps[:, w - 256:w], maskt[:, s, :]
                        )
                    nc.scalar.activation(
                        probs[:, lo:lo + w],
                        ps[:],
                        AF.Exp,
                        scale=SCALE,
                        accum_out=sums[:, c:c + 1],
                    )
                    # PE transpose: the PE has slack here, and keeping the
                    # ACT ring free lets exp drain PSUM without queueing
                    # behind XBAR transposes
                    for t in range(lo // P, (lo + w) // P):
                        pt = psT.tile([P, P], BF16, tag="tr")
                        nc.tensor.transpose(
                            pt[:], probs[:, t * P:(t + 1) * P], ident[:]
                        )
                        nc.vector.tensor_copy(probsT[:, t, :], pt[:])
                den = smallp.tile([P, 1], F32, tag="den")
                nc.vector.reduce_sum(den[:], sums[:, :nch], axis=AX.X)
                rec = smallp.tile([P, 1], F32, tag="rec")
                nc.vector.reciprocal(rec[:], den[:])
                pendings.append((s, probsT, rec, ltiles))
            for p_ in pendings:
                emit_av(*p_)
            nc.leave_named_scope("ph2", 0, notify=False)
    nc.compile()
    return nc


def _make_masks():
    masks = []
    for blocks in (BLOCKS_A, BLOCKS_B):
        m = np.zeros((P, 8, 256), np.float32)
        for s, j in enumerate(blocks):
            q = j * P + np.arange(P)[:, None]
            k = 256 * s + np.arange(256)[None, :]
            m[:, s, :] = np.where(k <= q, 0.0, NEG)
        masks.append(m)
    return masks


def _bf16(a):
    return np.ascontiguousarray(a.astype(ml_dtypes.bfloat16))


def _panelize(wT):
    # wT: [D, D] = [dt*128+p, et*128+es] -> [et, p, dt, es]
    return _bf16(wT.reshape(8, P, 8, P).transpose(2, 1, 0, 3))


def _chunk_panels(rows, nck):
    # [nck*512, 1024] -> [ck, p, dt, s] with [ck,p,dt,s] = rows[ck*512+s, dt*128+p]
    return _bf16(rows.reshape(nck, 512, 8, P).transpose(0, 3, 2, 1))


LAST_RESULT = None


def kernel(x, wq, wk, wv):
    global LAST_RESULT
    x = np.ascontiguousarray(np.asarray(x, dtype=np.float32))
    wq = np.asarray(wq, dtype=np.float32)
    wk = np.asarray(wk, dtype=np.float32)
    wv = np.asarray(wv, dtype=np.float32)

    if "nc" not in _CACHE:
        _CACHE["nc"] = _build()
        _CACHE["masks"] = _make_masks()
    nc = _CACHE["nc"]
    masks = _CACHE["masks"]

    # exact host-side fold: scores = xq (wq^T wk) x^T
    W = wq.T @ wk
    Wp = _panelize(W)
    # per-half wv rhs panels: [p, dt, e'] = wv[h*512+e', dt*128+p]
    wvhs = [
        _bf16(wv[h * EH:(h + 1) * EH].T.reshape(8, P, EH).transpose(1, 0, 2))
        for h in range(2)
    ]

    in_maps = []
    for c in range(8):
        b, h = divmod(c, 2)
        blocks = BLOCKS_A if h == 0 else BLOCKS_B
        xb = x[b]
        xq = np.concatenate([xb[j * P:(j + 1) * P] for j in blocks], 0)
        in_maps.append(
            {
                "xTP": _chunk_panels(xb, 4),
                "xqTP": _chunk_panels(xq, 2),
                "Wp": Wp,
                "wvh": wvhs[h],
                "maskb": masks[h],
            }
        )

    res = run_bass_kernel_spmd(nc, in_maps, core_ids=list(range(8)))
    LAST_RESULT = res

    out = np.empty((B, S, D), np.float32)
    for c in range(8):
        b, h = divmod(c, 2)
        blocks = BLOCKS_A if h == 0 else BLOCKS_B
        oc = res.results[c]["out"]
        for si, j in enumerate(blocks):
            out[b, j * P:(j + 1) * P] = oc[si * P:(si + 1) * P]
    return out


# revision 6
# speedup vs baseline: 1.0926x; 1.0391x over previous
"""Causal single-head attention (B=4, S=2048, D=1024) on 8 TRN2 NeuronCores.

Sharding: data-parallel, 2 cores per batch element; 16 query blocks split
between the pair in a triangle-balanced pattern (A: 0,3,4,7,8,11,12,15 /
B: the rest) so both cores see the same multiset of padded causal key
ranges.

Two algebraic restructurings kill most of the projection work:
  * scores = (xq wq^T)(x wk^T)^T = xq (wq^T wk) x^T. W = wq^T wk is folded
    on the host (exact, fp32), so the kernel computes U^T = W^T xq^T once
    (2.15 GF) and contracts scores directly against the x^T panels already
    resident in SBUF — the K projection (4.3 GF/batch) vanishes.
  * V is tensor-parallel within the pair on its OUTPUT columns: core h
    computes V[:, h*512:(h+1)*512] = x wv_half^T only (half the work), and
    the halves are exchanged with a single 2-core AllGather via HBM
    staging. Rank-order concat puts both halves in global e-order on both
    cores, so the per-core half selection lives entirely in the wv DATA.

Per core: 9.14 GFLOP, all matmuls bf16 with fp32 PSUM accumulation:
  V_mine = x @ wv_half^T  -> stage -> AllGather (hidden under UT + scores)
  U^T    = W^T @ xq^T
  all 8 score slots (largest first): scores(s) = U^T(s-block) . x^T, causal
    mask on the last 256 cols (additive, fed as data), exp with accumulated
    row sums (no max-subtraction: |scores| <= ~11), probs transposed on the
    PE (the ACT engine is saturated by exp; the PE has slack here),
  then all 8 AV slots: probs^T @ V -> scale by 1/rowsum -> out.

kernel(x, wq, wk, wv) takes full inputs, returns the full [4,2048,1024] output.
"""
import ml_dtypes
import numpy as np

import concourse.bacc as bacc
import concourse.mybir as mybir
import concourse.tile as tile
from concourse.bass_utils import run_bass_kernel_spmd
from concourse.masks import make_identity

F32 = mybir.dt.float32
BF16 = mybir.dt.bfloat16
AX = mybir.AxisListType
AF = mybir.ActivationFunctionType

P = 128
B, S, D = 4, 2048, 1024
NQ = 1024            # query rows per core
EH = 512             # V e-half computed locally per core
BLOCKS_A = [0, 3, 4, 7, 8, 11, 12, 15]
BLOCKS_B = [1, 2, 5, 6, 9, 10, 13, 14]
NEG = -1e30
SCALE = 1.0 / 32.0   # 1/sqrt(D)
PAIRS = [[0, 1], [2, 3], [4, 5], [6, 7]]

_CACHE = {}


def _build():
    nc = bacc.Bacc("TRN2", target_bir_lowering=False, debug=False, num_devices=8)
    # x chunks pre-paneled host-side: [ck, p, dt, s] = x[ck*512+s, dt*128+p]
    xTP_d = nc.declare_dram_parameter("xTP", [4, P, 8, 512], BF16, isOutput=False)
    xqTP_d = nc.declare_dram_parameter("xqTP", [2, P, 8, 512], BF16, isOutput=False)
    # folded-W lhsT panels: [et, p, dt, es] = W[dt*128+p, et*128+es]
    Wp_d = nc.declare_dram_parameter("Wp", [8, P, 8, P], BF16, isOutput=False)
    # wv half rhs panels: [p, dt, e'] = wv[h*512+e', dt*128+p]
    wvh_d = nc.declare_dram_parameter("wvh", [P, 8, EH], BF16, isOutput=False)
    mb_d = nc.declare_dram_parameter("maskb", [P, 8, 256], F32, isOutput=False)
    out_d = nc.declare_dram_parameter("out", [NQ, D], F32, isOutput=True)

    # HBM staging for the pairwise V e-half exchange
    v_stage = nc.dram_tensor("v_stage", [P, 16 * EH], BF16)
    v_gath = nc.dram_tensor("v_gath", [2 * P, 16 * EH], BF16)

    with tile.TileContext(nc) as tc:
        with (
            tc.tile_pool(name="store", bufs=1) as store,
            tc.tile_pool(name="whalf", bufs=1) as whalf,
            tc.tile_pool(name="kvmine", bufs=1) as kvmine,
            tc.tile_pool(name="wqpool", bufs=3) as wqpool,
            tc.tile_pool(name="xpool", bufs=6) as xpool,
            tc.tile_pool(name="prpool", bufs=10) as prpool,
            tc.tile_pool(name="small", bufs=4) as smallp,
            tc.tile_pool(name="outp", bufs=3) as outp,
            tc.tile_pool(name="ps_a", bufs=1, space="PSUM") as psA,
            tc.tile_pool(name="ps_b", bufs=5, space="PSUM") as psB,
            tc.tile_pool(name="ps_t", bufs=2, space="PSUM") as psT,
        ):
            ident = store.tile([P, P], BF16)
            make_identity(nc, ident[:])
            V = store.tile([P, 2, 16, EH], BF16)  # [k%128, e-half, k//128, e%512]
            UT = store.tile([P, 8, NQ], BF16)   # [e%128, e//128, q]

            # ---- input DMAs in first-use order ----
            wvh = whalf.tile([P, 8, EH], BF16, tag="wv")
            for dt in range(8):
                nc.scalar.dma_start(wvh[:, dt, :], wvh_d.ap()[:, dt, :])
            xts = []
            for ck in range(4):
                xt = xpool.tile([P, 8, 512], BF16, tag="x")
                if ck == 0:
                    for dt in range(8):
                        nc.sync.dma_start(xt[:, dt, :], xTP_d.ap()[ck][:, dt, :])
                else:
                    nc.sync.dma_start(xt[:, 0:4], xTP_d.ap()[ck][:, 0:4])
                    nc.sync.dma_start(xt[:, 4:8], xTP_d.ap()[ck][:, 4:8])
                xts.append(xt)

            # ---- V_mine = x @ wv_half^T  ([P, 16, EH], e-half h) ----
            nc.enter_named_scope("p1_v", notify=False)
            Vm = kvmine.tile([P, 16, EH], BF16, tag="v")
            for ck in range(4):
                xt = xts[ck]
                for sub in range(4):
                    ps = psB.tile([P, 512], F32, tag="pb")
                    for dt in range(8):
                        nc.tensor.matmul(
                            ps[:],
                            xt[:, dt, sub * P:(sub + 1) * P],
                            wvh[:, dt, :],
                            start=(dt == 0),
                            stop=(dt == 7),
                        )
                    nc.any.tensor_copy(Vm[:, ck * 4 + sub, :], ps[:])
                nc.scalar.dma_start(
                    v_stage[:, ck * 4 * EH:(ck + 1) * 4 * EH],
                    Vm[:, ck * 4:(ck + 1) * 4, :],
                )
            nc.leave_named_scope("p1_v", 0, notify=False)
            nc.gpsimd.collective_compute(
                "AllGather",
                mybir.AluOpType.bypass,
                replica_groups=PAIRS,
                ins=[v_stage[:].opt()],
                outs=[v_gath[:].opt()],
            )
            # unpack on the gpsimd (SWDGE) ring — it has nothing else to do,
            # and with a contiguous destination the descriptor count is tiny
            nc.gpsimd.dma_start(V[:, 0], v_gath[0:P, :])
            nc.gpsimd.dma_start(V[:, 1], v_gath[P:2 * P, :])

            # ---- UT = W^T @ xq^T (et-outer, streamed W panels) ----
            nc.enter_named_scope("p1_ut", notify=False)
            xqs = []
            for cq in range(2):
                xq = xpool.tile([P, 8, 512], BF16, tag="x")
                nc.sync.dma_start(xq[:], xqTP_d.ap()[cq])
                xqs.append(xq)
            maskt = store.tile([P, 8, 256], F32)
            for et in range(8):
                wqp = wqpool.tile([P, 8, P], BF16, tag="wq")
                # late panels ride the scalar ring (idle after V staging) so
                # they don't queue behind the x/xq bulk on the sync ring
                if et < 4:
                    nc.sync.dma_start(wqp[:], Wp_d.ap()[et])
                else:
                    nc.scalar.dma_start(wqp[:], Wp_d.ap()[et])
                if et == 0:
                    nc.sync.dma_start(maskt[:], mb_d.ap())
                for cq in range(2):
                    ps = psB.tile([P, 512], F32, tag="pb")
                    for dt in range(8):
                        nc.tensor.matmul(
                            ps[:],
                            wqp[:, dt, :],
                            xqs[cq][:, dt, :],
                            start=(dt == 0),
                            stop=(dt == 7),
                        )
                    nc.any.tensor_copy(UT[:, et, cq * 512:(cq + 1) * 512], ps[:])
            nc.leave_named_scope("p1_ut", 0, notify=False)

            # ---- phase 2: all score slots first (they only need x^T + UT),
            # then all AV slots — the V gather finishes under the scores. ----
            def emit_av(s, probsT, rec, ltiles):
                for ev in range(2):
                    pav = psB.tile([P, 512], F32, tag="pb")
                    for t in range(ltiles):
                        nc.tensor.matmul(
                            pav[:],
                            probsT[:, t, :],
                            V[:, ev, t, :],
                            start=(t == 0),
                            stop=(t == ltiles - 1),
                        )
                    ot = outp.tile([P, 512], F32, tag="out")
                    nc.scalar.activation(ot[:], pav[:], AF.Copy, scale=rec[:])
                    nc.sync.dma_start(
                        out_d.ap()[s * P:(s + 1) * P, ev * 512:(ev + 1) * 512],
                        ot[:],
                    )

            nc.enter_named_scope("ph2", notify=False)
            pendings = []
            for s in range(7, -1, -1):
                ltiles = 2 * (s + 1)           # 128-wide key tiles
                keys = 256 * (s + 1)
                n512 = (s + 1) // 2            # full 512-wide chunks
                rem = (s + 1) % 2              # one trailing 256-wide chunk?
                nch = n512 + rem
                probs = prpool.tile([P, S], BF16, tag="pr")
                probsT = prpool.tile([P, 16, P], BF16, tag="pr")
                sums = smallp.tile([P, 8], F32, tag="sums")
                for c in range(nch):
                    is_rem = rem and c == nch - 1
                    w = 256 if is_rem else 512
                    lo = c * 512
                    if is_rem:
                        ps = psA.tile([P, 256], F32, tag="pa")
                    else:
                        ps = psB.tile([P, 512], F32, tag="pb")
                    for dt in range(8):
                        nc.tensor.matmul(
                            ps[:],
                            UT[:, dt, s * P:(s + 1) * P],
                            xts[c][:, dt, 0:w],
                            start=(dt == 0),
                            stop=(dt == 7),
                        )
                    if lo + w == keys:   # mask the last 256 key columns
                        nc.vector.tensor_add(
                            ps[:, w - 256:w], ps[:, w - 256:w], maskt[:, s, :]
                        )
                    nc.scalar.activation(
                        probs[:, lo:lo + w],
                        ps[:],
                        AF.Exp,
                        scale=SCALE,
                        accum_out=sums[:, c:c + 1],
                    )
                    # PE transpose: the PE has slack here, and keeping the
                    # ACT ring free lets exp drain PSUM without queueing
                    # behind XBAR transposes
                    for t in range(lo // P, (lo + w) // P):
                        pt = psT.tile([P, P], BF16, tag="tr")
                        nc.tensor.transpose(
                            pt[:], probs[:, t * P:(t + 1) * P], ident[:]
                        )
                        nc.vector.tensor_copy(probsT[:, t, :], pt[:])
                den = smallp.tile([P, 1], F32, tag="den")
                nc.vector.reduce_sum(den[:], sums[:, :nch], axis=AX.X)
                rec = smallp.tile([P, 1], F32, tag="rec")
                nc.vector.reciprocal(rec[:], den[:])
                pendings.append((s, probsT, rec, ltiles))
            for p_ in pendings:
                emit_av(*p_)
            nc.leave_named_scope("ph2", 0, notify=False)
    nc.compile()
    return nc


def _make_masks():
    masks = []
    for blocks in (BLOCKS_A, BLOCKS_B):
        m = np.zeros((P, 8, 256), np.float32)
        for s, j in enumerate(blocks):
            q = j * P + np.arange(P)[:, None]
            k = 256 * s + np.arange(256)[None, :]
            m[:, s, :] = np.where(k <= q, 0.0, NEG)
        masks.append(m)
    return masks


def _bf16(a):
    return np.ascontiguousarray(a.astype(ml_dtypes.bfloat16))


def _panelize(wT):
    # wT: [D, D] = [dt*128+p, et*128+es] -> [et, p, dt, es]
    return _bf16(wT.reshape(8, P, 8, P).transpose(2, 1, 0, 3))


def _chunk_panels(rows, nck):
    # [nck*512, 1024] -> [ck, p, dt, s] with [ck,p,dt,s] = rows[ck*512+s, dt*128+p]
    return _bf16(rows.reshape(nck, 512, 8, P).transpose(0, 3, 2, 1))


LAST_RESULT = None


def kernel(x, wq, wk, wv):
    global LAST_RESULT
    x = np.ascontiguousarray(np.asarray(x, dtype=np.float32))
    wq = np.asarray(wq, dtype=np.float32)
    wk = np.asarray(wk, dtype=np.float32)
    wv = np.asarray(wv, dtype=np.float32)

    if "nc" not in _CACHE:
        _CACHE["nc"] = _build()
        _CACHE["masks"] = _make_masks()
    nc = _CACHE["nc"]
    masks = _CACHE["masks"]

    # exact host-side fold: scores = xq (wq^T wk) x^T
    W = wq.T @ wk
    Wp = _panelize(W)
    # per-half wv rhs panels: [p, dt, e'] = wv[h*512+e', dt*128+p]
    wvhs = [
        _bf16(wv[h * EH:(h + 1) * EH].T.reshape(8, P, EH).transpose(1, 0, 2))
        for h in range(2)
    ]

    in_maps = []
    for c in range(8):
        b, h = divmod(c, 2)
        blocks = BLOCKS_A if h == 0 else BLOCKS_B
        xb = x[b]
        xq = np.concatenate([xb[j * P:(j + 1) * P] for j in blocks], 0)
        in_maps.append(
            {
                "xTP": _chunk_panels(xb, 4),
                "xqTP": _chunk_panels(xq, 2),
                "Wp": Wp,
                "wvh": wvhs[h],
                "maskb": masks[h],
            }
        )

    res = run_bass_kernel_spmd(nc, in_maps, core_ids=list(range(8)))
    LAST_RESULT = res

    out = np.empty((B, S, D), np.float32)
    for c in range(8):
        b, h = divmod(c, 2)
        blocks = BLOCKS_A if h == 0 else BLOCKS_B
        oc = res.results[c]["out"]
        for si, j in enumerate(blocks):
            out[b, j * P:(j + 1) * P] = oc[si * P:(si + 1) * P]
    return out
